# revision 64
# baseline (speedup 1.0000x reference)
"""Causal single-head attention (B=4, S=4096, E=1024, H=128) on 8 TRN2 NeuronCores.

Sharding: 8 cores = 4 batches x 2 sequence shards. Each core handles 4 query
blocks of 512 rows of one batch. Causal work per q-block j is 4*(j+1) k-tiles
(128 keys each); blocks are split {7,5,2,0} / {6,4,3,1} so both shards cost 72
k-tiles, padded to a uniform program of [32,24,16,8] k-tiles per slot so all 8
cores run one SPMD program.

Per core, one interleaved pipeline:
  - K^T/V^T projection blocks (bf16 matmuls, fp32 PSUM) streamed off chunked
    embT DMAs, V transposed on the PE, followed by flash-style attention in
    scores-transposed layout (scores^T = K_tile^T.T @ Q^T), exp on ACT,
    causal masking via fused DVE tensor_paged_mask driven by a tiny per-core
    threshold tensor (no mask DMA), AV accumulated in PSUM.
  - Q^T projections and output epilogues are emitted between attention pairs
    as PE filler work so the tensor engine never idles.
  - Softmax denominator: DVE/GPSIMD accumulate exp partials; tiny
    [128,1]-output matmuls reduce them per 128-column chunk into column form,
    so normalization is a per-partition scalar multiply fused with the
    PSUM->SBUF copy after the PE output transpose.
  - ~3us of tiny warm-up matmuls at t=0 keep the PE p-state ramp warm through
    the first DMA latency.
"""

import numpy as np
import ml_dtypes

import concourse.bacc as bacc
import concourse.bass as bass
import concourse.mybir as mybir
import concourse.tile as tile
from concourse.bass_utils import run_bass_kernel_spmd
from concourse.masks import make_identity

BF16 = ml_dtypes.bfloat16
F32 = np.float32

B, S, E, H = 4, 4096, 1024, 128
NCORES = 8
PROG = [32, 24, 16, 8]                       # program k-tile count per slot
BLOCKS = {0: [7, 5, 2, 0], 1: [6, 4, 3, 1]}  # parity -> owned q-block ids
EC = E // 128                                 # 8 contraction chunks
SB = S // 512                                 # 8 key blocks of 512
QB = 4                                        # q-blocks (slots) per core
QLEN = QB * 512                               # 2048 q rows per core

_CACHE = {}


def _build_program():
    dt = mybir.dt
    nc = bacc.Bacc("TRN2", target_bir_lowering=False, debug=False, num_devices=NCORES)

    embT_d = nc.dram_tensor("embT", [E, S], dt.bfloat16, kind="ExternalInput")
    embTq_d = nc.dram_tensor("embTq", [E, QLEN], dt.bfloat16, kind="ExternalInput")
    wpack_d = nc.dram_tensor("wpack", [128, 3, EC, H], dt.bfloat16, kind="ExternalInput")
    bpack_d = nc.dram_tensor("bpack", [128, 3], dt.float32, kind="ExternalInput")
    mask_d = nc.dram_tensor("maskblk", [QB, 128, 8, 512], dt.bfloat16, kind="ExternalInput")
    out_d = nc.dram_tensor("out", [QLEN, H], dt.float32, kind="ExternalOutput")

    ident_f = mybir.ActivationFunctionType.Identity
    exp_f = mybir.ActivationFunctionType.Exp

    with tile.TileContext(nc) as tc:
        with tc.tile_pool(name="singles", bufs=1) as singles, \
             tc.tile_pool(name="etp", bufs=3) as etp, \
             tc.tile_pool(name="qetp", bufs=2) as qetp, \
             tc.tile_pool(name="mtp", bufs=3) as mtp, \
             tc.tile_pool(name="ptp", bufs=9) as ptp, \
             tc.tile_pool(name="paccp", bufs=2) as paccp, \
             tc.tile_pool(name="ocp", bufs=2) as ocp, \
             tc.tile_pool(name="posp", bufs=2) as posp, \
             tc.tile_pool(name="recp", bufs=2) as recp, \
             tc.tile_pool(name="qhap", bufs=2) as qhap, \
             tc.tile_pool(name="psp", bufs=2, space="PSUM") as psp, \
             tc.tile_pool(name="pop", bufs=2, space="PSUM") as pop, \
             tc.tile_pool(name="auxp", bufs=2, space="PSUM") as auxp:

            # ---- constants / weights (front of the SP DMA queue) ----
            wpack = singles.tile([128, 3, EC, H], dt.bfloat16, tag="wpack")
            nc.sync.dma_start(out=wpack[:, 0, 0:1, :], in_=wpack_d.ap()[:, 0, 0:1])
            nc.sync.dma_start(out=wpack[:, 0, 1:, :], in_=wpack_d.ap()[:, 0, 1:])
            bp = singles.tile([128, 3], dt.float32, tag="bp")
            nc.sync.dma_start(out=bp[:, :], in_=bpack_d.ap())
            nc.sync.dma_start(out=wpack[:, 1, :, :], in_=wpack_d.ap()[:, 1])

            # warm-up source available almost immediately (Pool memsets start
            # first after the entry barrier)
            wsrc = singles.tile([128, 64], dt.bfloat16, tag="wsrc")
            nc.gpsimd.memset(wsrc[:, :], 0.5)
            identb = singles.tile([128, 128], dt.bfloat16, tag="identb")
            make_identity(nc, identb[:, :])
            identf = singles.tile([128, 128], dt.float32, tag="identf")
            make_identity(nc, identf[:, :])
            ones_h = singles.tile([128, 1], dt.float16, tag="ones_h")
            nc.vector.memset(ones_h[:, :], 1.0)
            ones_b = singles.tile([128, 1], dt.bfloat16, tag="ones_b")
            nc.vector.memset(ones_b[:, :], 1.0)

            # ---- PE warm-up: keep the p-state ramp alive through DMA latency ----
            for i in range(62):
                wm = auxp.tile([128, 64], dt.float32, tag="aux", name=f"wm{i}")
                nc.tensor.matmul(wm[:64, :], lhsT=wsrc[:, :], rhs=wsrc[:, :],
                                 start=True, stop=True)

            kTs = [singles.tile([128, 512], dt.bfloat16, tag=f"kT{i}", name=f"kT{i}")
                   for i in range(SB)]
            vTs = [singles.tile([128, 512], dt.bfloat16, tag=f"vT{i}", name=f"vT{i}")
                   for i in range(SB)]
            vts = [singles.tile([128, 128], dt.bfloat16, tag=f"v{i}", name=f"v{i}")
                   for i in range(4 * SB)]
            qTs = [singles.tile([128, 512], dt.bfloat16, tag=f"qT{i}", name=f"qT{i}")
                   for i in range(QB)]

            pending_tr = []  # deferred V-transposes: (st, vT tile, u)

            def flush_tr(n=1000):
                # one PE transpose + Pool copy per call site; spaced so the
                # 2-slot aux rotation never stalls the PE on the copy
                for _ in range(min(n, len(pending_tr))):
                    st, vT, u = pending_tr.pop(0)
                    tp = auxp.tile([128, 128], dt.bfloat16, tag="aux", name=f"tp{st}")
                    nc.tensor.transpose(tp[:, :], vT[:, 128 * u:128 * (u + 1)],
                                        identb[:, :])
                    nc.vector.tensor_copy(vts[st][:, :], tp[:, :])

            def kv_dma(sb):
                et = etp.tile([128, EC, 512], dt.bfloat16, tag="et", name=f"et{sb}")
                # block 0 streams in single-chunk DMAs up front to cut the
                # first-matmul latency
                spans = [(0, 1), (1, 2), (2, 4), (4, 6), (6, 8)] if sb == 0 else \
                        [(0, 2), (2, 4), (4, 6), (6, 8)]
                src = embT_d.ap().rearrange("(c p) s -> p c s", p=128)
                for c0, c1 in spans:
                    nc.sync.dma_start(
                        out=et[:, c0:c1, :],
                        in_=src[:, c0:c1, 512 * sb:512 * (sb + 1)],
                    )
                return et

            def kv_block(sb, et=None):
                if et is None:
                    et = kv_dma(sb)
                ps = psp.tile([128, 2, 512], dt.float32, tag="ps", name=f"pskv{sb}")
                for c in range(EC):
                    nc.tensor.matmul(ps[:, 0, :], lhsT=wpack[:, 0, c, :], rhs=et[:, c, :],
                                     start=(c == 0), stop=(c == EC - 1))
                    nc.tensor.matmul(ps[:, 1, :], lhsT=wpack[:, 1, c, :], rhs=et[:, c, :],
                                     start=(c == 0), stop=(c == EC - 1))
                    if c >= 3 and sb <= 4:
                        flush_tr(1)  # blocks 0-3's V transposes, spaced two
                        # chunks apart for the single-slot aux rotation;
                        # blocks 4-7 become attention-phase fillers
                nc.scalar.activation(kTs[sb][:, :], ps[:, 0, :], ident_f, bias=bp[:, 0:1])
                vT = vTs[sb]
                nc.scalar.activation(vT[:, :], ps[:, 1, :], ident_f, bias=bp[:, 1:2])
                pending_tr.extend((4 * sb + u, vT, u) for u in range(4))

            # filler units: emitted between attention pairs to keep PE dense
            fillers = []

            def emit_fillers(n):
                for _ in range(n):
                    if fillers:
                        fillers.pop(0)()

            qets = {}

            def qet_load(s):
                qet = qetp.tile([128, EC, 512], dt.bfloat16, tag="qet", name=f"qet{s}")
                src = embTq_d.ap().rearrange("(c p) s -> p c s", p=128)
                for c0 in range(0, EC, 2):
                    nc.sync.dma_start(out=qet[:, c0:c0 + 2, :],
                                      in_=src[:, c0:c0 + 2, 512 * s:512 * (s + 1)])
                qets[s] = qet

            def qproj(s):
                qet = qets.pop(s)
                ps = psp.tile([128, 2, 512], dt.float32, tag="ps", name=f"psq{s}")
                for c in range(EC):
                    nc.tensor.matmul(ps[:, 0, :], lhsT=wpack[:, 2, c, :],
                                     rhs=qet[:, c, :],
                                     start=(c == 0), stop=(c == EC - 1))
                # bias on DVE: ACT is saturated with exps at slot transitions
                nc.vector.tensor_scalar_add(qTs[s][:, :], ps[:, 0, :], bp[:, 2:3])
                # parity-restoring dummy allocation (never written -> free)
                psp.tile([128, 2, 512], dt.float32, tag="ps", name=f"psd{s}")

            qhas = {}

            def qproj_half(s, half):
                """Q projection split into two 4-matmul bursts, each shorter
                than one exp so the ACT stream never starves; halves joined
                via an SBUF fp32 partial."""
                qet = qets[s] if half == 0 else qets.pop(s)
                ps = psp.tile([128, 2, 512], dt.float32, tag="ps",
                              name=f"psq{s}_{half}")
                for c in range(4 * half, 4 * half + 4):
                    nc.tensor.matmul(ps[:, 0, :], lhsT=wpack[:, 2, c, :],
                                     rhs=qet[:, c, :],
                                     start=(c == 4 * half), stop=(c == 4 * half + 3))
                if half == 0:
                    qha = qhap.tile([128, 512], dt.float32, tag="qha", name=f"qha{s}")
                    nc.vector.tensor_scalar_add(qha[:, :], ps[:, 0, :], bp[:, 2:3])
                    qhas[s] = qha
                else:
                    nc.vector.tensor_add(qTs[s][:, :], ps[:, 0, :], qhas.pop(s)[:, :])
                psp.tile([128, 2, 512], dt.float32, tag="ps", name=f"psd{s}_{half}")

            slot_state = {}

            def sc(s, p):
                st = slot_state[s]
                NP = PROG[s] // 2
                ps = psp.tile([128, 2, 512], dt.float32, tag="ps", name=f"ps{s}_{p}")
                for h2 in (0, 1):
                    t = 2 * p + h2
                    nc.tensor.matmul(ps[:, h2, :],
                                     lhsT=kTs[t // 4][:, 128 * (t % 4):128 * (t % 4 + 1)],
                                     rhs=qTs[s][:, :], start=True, stop=True)
                pt = ptp.tile([128, 2, 512], dt.bfloat16, tag="pt", name=f"pt{s}_{p}")
                nc.scalar.activation(pt[:, :, :], ps[:, :, :], exp_f)
                masked = p >= NP - 4
                if masked:
                    pp = p - (NP - 4)
                    mt = st["mt"]
                    # causal mask multiplies, split DVE/Pool to stay under the
                    # ACT exp cadence; these pairs feed the denominator via
                    # direct column-sum matmuls instead of pacc adds. The last
                    # pairs of the final slot stay on DVE (short chain) so the
                    # slow Pool chain doesn't pad the drain tail.
                    eng1 = nc.vector if (s == 0 and p >= NP - 2) else nc.gpsimd
                    nc.vector.tensor_mul(pt[:, 0, :], pt[:, 0, :], mt[:, 2 * pp, :])
                    eng1.tensor_mul(pt[:, 1, :], pt[:, 1, :], mt[:, 2 * pp + 1, :])
                    st["mpts"][p] = pt
                elif p == 0:
                    nc.vector.tensor_copy(st["pa"][:, :], pt[:, 0, :])
                    nc.vector.tensor_copy(st["pb"][:, :], pt[:, 1, :])
                else:
                    nc.vector.tensor_add(st["pa"][:, :], st["pa"][:, :], pt[:, 0, :])
                    nc.vector.tensor_add(st["pb"][:, :], st["pb"][:, :], pt[:, 1, :])
                st["pts"][p] = pt

            def av(s, p):
                st = slot_state[s]
                Wp = PROG[s]
                pt = st["pts"].pop(p)
                for h2 in (0, 1):
                    t = 2 * p + h2
                    nc.tensor.matmul(st["po"][:, :], lhsT=vts[t][:, :], rhs=pt[:, h2, :],
                                     start=(t == 0), stop=(t == Wp - 1))

            def epilogue(s, last):
                st = slot_state[s]
                NP = PROG[s] // 2
                # head: column-form denominator (inline, cheap PE matmuls over
                # the fp16 accumulators plus the masked pairs' pt tiles)
                lc = auxp.tile([128, 4], dt.float32, tag="aux", name=f"lc{s}")
                for u in range(4):
                    terms = []
                    if NP > 4:
                        terms += [(st["pa"], ones_h), (st["pb"], ones_h)]
                    for p in sorted(st["mpts"]):
                        pt = st["mpts"][p]
                        terms += [(pt[:, 0, :], ones_b), (pt[:, 1, :], ones_b)]
                    for j, (src, ones) in enumerate(terms):
                        nc.tensor.matmul(lc[:, u:u + 1],
                                         lhsT=src[:, 128 * u:128 * (u + 1)],
                                         rhs=ones[:, :], start=(j == 0),
                                         stop=(j == len(terms) - 1))
                rec = recp.tile([128, 4], dt.float32, tag="rec", name=f"rec{s}")
                nc.vector.reciprocal(rec[:, :], lc[:, :])
                pos = posp.tile([128, 512], dt.float32, tag="pos", name=f"pos{s}")
                if last:
                    # ACT is idle in the drain tail; run the copy there in
                    # parallel with the reciprocal on DVE
                    nc.scalar.activation(pos[:, :], st["po"][:, :], ident_f)
                else:
                    nc.vector.tensor_copy(pos[:, :], st["po"][:, :])
                # tail: transpose + normalize-on-copy + store, as fillers
                oc = ocp.tile([128, 4, 128], dt.float32, tag="oc", name=f"oc{s}")

                def tr_piece(u):
                    def emit():
                        # spread the last epilogue's transposes over distinct
                        # PSUM pools so they don't serialize on the aux rotation
                        if last and u == 1:
                            tro = pop.tile([128, 128], dt.float32, tag="po",
                                           name=f"tro{s}_{u}")
                        elif last and u == 2:
                            tro = psp.tile([128, 128], dt.float32, tag="ps",
                                           name=f"tro{s}_{u}")
                        else:
                            tro = auxp.tile([128, 128], dt.float32, tag="aux",
                                            name=f"tro{s}_{u}")
                        nc.tensor.transpose(tro[:, :], pos[:, 128 * u:128 * (u + 1)],
                                            identf[:, :])
                        if last and u % 2 == 1:
                            # normalize on the idle ACT engine in the tail so
                            # the four multiplies don't serialize on DVE
                            nc.scalar.activation(oc[:, u, :], tro[:, :], ident_f,
                                                 scale=rec[:, u:u + 1])
                        else:
                            nc.vector.tensor_scalar_mul(oc[:, u, :], tro[:, :],
                                                        rec[:, u:u + 1])
                        if last:
                            # one batched store; four serial HWDGE+DGE setups
                            # would pad the drain tail
                            if u == 3:
                                nc.sync.dma_start(
                                    out=out_d.ap()[512 * s:512 * (s + 1), :]
                                    .rearrange("(u p) h -> p u h", p=128),
                                    in_=oc[:, :, :],
                                )
                        else:
                            nc.sync.dma_start(
                                out=out_d.ap()[512 * s + 128 * u:512 * s + 128 * (u + 1), :],
                                in_=oc[:, u, :],
                            )
                    return emit

                if last:
                    for u in range(4):
                        tr_piece(u)()
                else:
                    fillers.extend(tr_piece(u) for u in range(4))

            # ---- pipeline ----
            for sb in range(SB - 1):
                kv_block(sb)
                if sb == 3:  # Q weights, needed from qproj(2) at ~30us
                    nc.sync.dma_start(out=wpack[:, 2, :, :], in_=wpack_d.ap()[:, 2])
            # first q-block's embeddings jump the DMA queue ahead of et7, and
            # its projection runs on the PE while kv7 waits for et7 chunks
            qet_load(2)
            et7 = kv_dma(7)
            qproj(2)
            kv_block(7, et=et7)

            # V-transposes for blocks 4-7 become PE fillers inside attention
            def tr_filler():
                def emit():
                    flush_tr(1)
                return emit
            fillers.extend(tr_filler() for _ in range(len(pending_tr)))

            mt_tiles = {}

            def mask_load(s, half):
                if half == 0:
                    mt_tiles[s] = mtp.tile([128, 8, 512], dt.bfloat16, tag="mt",
                                           name=f"mt{s}")
                nc.sync.dma_start(
                    out=mt_tiles[s][:, 4 * half:4 * (half + 1), :],
                    in_=mask_d.ap()[s, :, 4 * half:4 * (half + 1), :])

            def prep_slot(s):
                """qproj + first mask half for slot s"""
                qproj(s)
                mask_load(s, 0)

            # attention: one global software pipeline across all four slots;
            # scores run 4 pairs ahead of the AV matmuls so the masked pairs'
            # multiply chains never block the PE, and later slots' qprojs and
            # mask/q DMAs are prefetched at fixed points chosen against the
            # DMA stream's occupancy. Slot 2 first so slot 3's mask DMA has
            # headroom.
            SLOT_ORDER = (2, 3, 1, 0)
            units = [(s, p) for s in SLOT_ORDER for p in range(PROG[s] // 2)]
            LAG = 4
            qet_load(3)
            mask_load(2, 0)
            # prep points are spaced so one qproj's pieces (sharing the single
            # psq psum slot) fully drain before the next qproj starts
            POINTS = {
                (2, 1): [lambda: mask_load(2, 1), lambda: prep_slot(3)],
                (2, 2): [lambda: qet_load(1)],
                (2, 7): [lambda: mask_load(3, 1), lambda: prep_slot(1)],
                (3, 0): [lambda: qet_load(0)],
                (1, 1): [lambda: mask_load(1, 1), lambda: prep_slot(0)],
                (0, 1): [lambda: mask_load(0, 1)],
            }
            for i in range(len(units) + LAG):
                if i >= LAG:
                    s2, p2 = units[i - LAG]
                    av(s2, p2)
                    emit_fillers(1)
                    if p2 == PROG[s2] // 2 - 1:
                        epilogue(s2, last=(s2 == 0))
                if i < len(units):
                    s, p = units[i]
                    if p == 0:
                        slot_state[s] = {
                            "po": pop.tile([128, 512], dt.float32, tag="po",
                                           name=f"po{s}"),
                            "mt": mt_tiles[s],
                            "pts": {},
                            "mpts": {},
                        }
                        if PROG[s] // 2 > 4:
                            slot_state[s]["pa"] = paccp.tile(
                                [128, 512], dt.float16, tag="pacc_a", name=f"pacc_a{s}")
                            slot_state[s]["pb"] = paccp.tile(
                                [128, 512], dt.float16, tag="pacc_b", name=f"pacc_b{s}")
                    sc(s, p)
                    for fn in POINTS.get((s, p), ()):
                        fn()
            emit_fillers(len(fillers))

    nc.compile()
    return nc


def _build_maskblk(parity):
    m = np.zeros((QB, 128, 8, 512), np.float32)
    kk = np.arange(128)[:, None]
    qq = np.arange(512)[None, :]
    for s, j in enumerate(BLOCKS[parity]):
        Wp, Wa = PROG[s], 4 * (j + 1)
        for i in range(8):
            d = (Wp - 8 + i) - (Wa - 4)
            m[s, :, i, :] = ((qq - 128 * d) >= kk)
    return m.astype(BF16)


def kernel(embds, Wq, bq, Wk, bk, Wv, bv):
    embds = np.asarray(embds, F32)
    Wq = np.asarray(Wq, F32); bq = np.asarray(bq, F32)
    Wk = np.asarray(Wk, F32); bk = np.asarray(bk, F32)
    Wv = np.asarray(Wv, F32); bv = np.asarray(bv, F32)

    if "nc" not in _CACHE:
        _CACHE["nc"] = _build_program()
    nc = _CACHE["nc"]

    scale = F32(1.0 / np.sqrt(H))

    def to_lhsT(w):
        return np.ascontiguousarray(
            w.astype(BF16).reshape(EC, 128, H).transpose(1, 0, 2))

    wpack = np.ascontiguousarray(
        np.stack([to_lhsT(Wk), to_lhsT(Wv), to_lhsT(Wq * scale)], axis=1))
    bpack = np.ascontiguousarray(np.stack([bk, bv, bq * scale], axis=1).astype(F32))
    masks = {p: _build_maskblk(p) for p in (0, 1)}

    embT = {b: np.ascontiguousarray(embds[b].T).astype(BF16) for b in range(B)}

    in_maps = []
    for c in range(NCORES):
        b, parity = c // 2, c % 2
        et = embT[b]
        etq = np.concatenate([et[:, 512 * j:512 * (j + 1)] for j in BLOCKS[parity]], axis=1)
        in_maps.append({
            "embT": et,
            "embTq": np.ascontiguousarray(etq),
            "wpack": wpack,
            "bpack": bpack,
            "maskblk": masks[parity],
        })

    res = run_bass_kernel_spmd(nc, in_maps, list(range(NCORES)))

    out = np.empty((B, S, H), F32)
    for c in range(NCORES):
        b, parity = c // 2, c % 2
        oc = res.results[c]["out"]
        for s, j in enumerate(BLOCKS[parity]):
            out[b, 512 * j:512 * (j + 1)] = oc[512 * s:512 * (s + 1)]
    return out
